# revision 1
# baseline (speedup 1.0000x reference)
"""AttentionJacobian kernel for 8 TRN2 NeuronCores.

J[b,q] = scale * ( V^T diag(a_q) K  -  (A V)_q ((A K)_q)^T ),  a = softmax(Q K^T scale)

Data-parallel over batch: 16 batches -> 2 per core. Per batch on-device:
  scoresT chunks (n x q) = KT_c^T @ QT      (f32 matmuls, exact)
  E = exp(scale * scoresT)                  (ScalarE, bf16 out; no max-sub needed:
                                             |scale*s| <~ 6 for randn inputs)
  Zrow = ones^T @ E                         (PE cross-partition sum)
  rzb = SCALE/Z broadcast via ones-matmul   (PE)
  AT = E * rzb                              (normalized, scaled A^T, bf16)
  per (q, chunk): SK = K_c * AT_col         (tensor_scalar on DVE/ACT/POOL)
  term1 psum += V_c^T @ SK                  (bf16 matmuls, f32 psum)
  OW/OT matmuls give w rows / o columns; term2 fused into psum evacuation:
  J = (w_bcast * (-o_col)) + psum           (scalar_tensor_tensor)
"""

import sys

for p in ("/opt/trn_rl_repo",):
    if p not in sys.path:
        sys.path.append(p)

import numpy as np
import ml_dtypes

import concourse.bass as bass
import concourse.bacc as bacc
import concourse.tile as tile
from concourse import mybir
from concourse.bass_utils import run_bass_kernel_spmd

N_CORES = 8
BATCH = 16
NQ = 64
SEQ = 4096
D = 128
BPC = BATCH // N_CORES        # batches per core = 2
C = SEQ // 128                # 32 contraction chunks
QG = 8                        # q per output group
NG = NQ // QG                 # 8 groups
SCALE = float(D) ** -0.5

F32 = mybir.dt.float32
BF16 = mybir.dt.bfloat16
AF = mybir.ActivationFunctionType
ALU = mybir.AluOpType

_CACHED = {}


def _build():
    nc = bacc.Bacc("TRN2", target_bir_lowering=False, debug=False,
                   num_devices=N_CORES)

    kvb = nc.dram_tensor("kvb", [BPC, C, 128, 256], BF16, kind="ExternalInput").ap()
    kt = nc.dram_tensor("kt", [BPC, 128, SEQ], F32, kind="ExternalInput").ap()
    qt = nc.dram_tensor("qt", [BPC, 128, NQ], F32, kind="ExternalInput").ap()
    out = nc.dram_tensor("out", [BPC, NQ, D, D], F32, kind="ExternalOutput").ap()

    with tile.TileContext(nc) as tc:
        with (
            tc.tile_pool(name="const", bufs=1) as constp,
            tc.tile_pool(name="kv", bufs=2) as kvp,
            tc.tile_pool(name="ktp", bufs=2) as ktp,
            tc.tile_pool(name="qtp", bufs=2) as qtp,
            tc.tile_pool(name="ep", bufs=2) as ep,
            tc.tile_pool(name="atp", bufs=2) as atp,
            tc.tile_pool(name="rzp", bufs=2) as rzp,
            tc.tile_pool(name="owp", bufs=2) as owp,
            tc.tile_pool(name="skp", bufs=4) as skp,
            tc.tile_pool(name="wbp", bufs=4) as wbp,
            tc.tile_pool(name="jsbp", bufs=3) as jsbp,
            tc.tile_pool(name="psj", bufs=2, space="PSUM") as psjp,
            tc.tile_pool(name="pss", bufs=2, space="PSUM") as pssp,
            tc.tile_pool(name="psmall", bufs=1, space="PSUM") as psmp,
        ):
            onescol = constp.tile([128, 1], BF16)
            nc.vector.memset(onescol[:, :], 1.0)
            onesrowS = constp.tile([1, 128], F32)
            nc.vector.memset(onesrowS[:, :], SCALE)

            sk_ctr = [0]

            def sk_engine():
                i = sk_ctr[0] % 32
                sk_ctr[0] += 1
                if i < 19:
                    return nc.vector
                if i < 25:
                    return nc.scalar
                return nc.gpsimd

            for b in range(BPC):
                KV = kvp.tile([128, C * 256], BF16, tag="kv")
                nc.sync.dma_start(KV[:, :].rearrange("p (c j) -> p c j", j=256),
                                  kvb[b].rearrange("c n j -> n c j"))
                KT = ktp.tile([128, SEQ], F32, tag="kt")
                nc.sync.dma_start(KT[:, :], kt[b])
                QT = qtp.tile([128, NQ], F32, tag="qt")
                nc.sync.dma_start(QT[:, :], qt[b])

                # --- softmax numerator: E = exp(scale * K Q^T), chunk by chunk
                E = ep.tile([128, C * NQ], BF16, tag="e")
                for c in range(C):
                    ps_s = pssp.tile([128, NQ], F32, tag="scores")
                    nc.tensor.matmul(ps_s[:, :], KT[:, c * 128:(c + 1) * 128],
                                     QT[:, :], start=True, stop=True)
                    nc.scalar.activation(E[:, c * NQ:(c + 1) * NQ], ps_s[:, :],
                                         AF.Exp, bias=0.0, scale=SCALE)

                # --- Z (1 x NQ) = ones^T E ; rzb = SCALE/Z broadcast to 128 parts
                ps_z = psmp.tile([128, NQ], F32, tag="small")
                for c in range(C):
                    nc.tensor.matmul(ps_z[0:1, :], onescol[:, :],
                                     E[:, c * NQ:(c + 1) * NQ],
                                     start=(c == 0), stop=(c == C - 1))
                rz = rzp.tile([1, NQ], F32, tag="rz")
                nc.vector.reciprocal(rz[:, :], ps_z[0:1, :])
                ps_rzb = psmp.tile([128, NQ], F32, tag="small")
                nc.tensor.matmul(ps_rzb[:, :], onesrowS[:, :], rz[:, :],
                                 start=True, stop=True)

                # --- AT = E * rzb  (chunk-major layout [c*NQ + q]); bf16 copy
                # for matmul lhsT use, f32 copy for tensor_scalar scalar reads
                rzbsb = rzp.tile([128, NQ], F32, tag="rzbsb")
                nc.scalar.copy(rzbsb[:, :], ps_rzb[:, :])
                AT = atp.tile([128, C * NQ], BF16, tag="at")
                nc.vector.tensor_mul(
                    AT[:, :].rearrange("p (c q) -> p c q", q=NQ),
                    E[:, :].rearrange("p (c q) -> p c q", q=NQ),
                    rzbsb[:, :].unsqueeze(1).broadcast_to((128, C, NQ)),
                )
                ATf = atp.tile([128, C * NQ], F32, tag="atf")
                nc.gpsimd.tensor_mul(
                    ATf[:, :].rearrange("p (c q) -> p c q", q=NQ),
                    E[:, :].rearrange("p (c q) -> p c q", q=NQ),
                    rzbsb[:, :].unsqueeze(1).broadcast_to((128, C, NQ)),
                )

                # --- w rows (q-partition) and o columns (v-partition)
                ps_ow = psmp.tile([128, NQ * 2], F32, tag="small2")
                for c in range(C):
                    nc.tensor.matmul(ps_ow[0:NQ, 0:128],
                                     AT[:, c * NQ:(c + 1) * NQ],
                                     KV[:, c * 256 + 128:(c + 1) * 256],
                                     start=(c == 0), stop=(c == C - 1))
                wsb = owp.tile([NQ, 128], F32, tag="wsb")
                nc.scalar.copy(wsb[:, :], ps_ow[0:NQ, 0:128])

                ps_ot = psmp.tile([128, NQ * 2], F32, tag="small2")
                for c in range(C):
                    nc.tensor.matmul(ps_ot[:, 0:NQ],
                                     KV[:, c * 256:c * 256 + 128],
                                     AT[:, c * NQ:(c + 1) * NQ],
                                     start=(c == 0), stop=(c == C - 1))
                negO = owp.tile([128, NQ], F32, tag="nego")
                nc.vector.tensor_scalar_mul(negO[:, :], ps_ot[:, 0:NQ],
                                            -1.0 / SCALE)

                # --- term1 + fused term2 per group of QG q's
                for g in range(NG):
                    ps_j = psjp.tile([128, QG * 128], F32, tag="j")
                    for c in range(C):
                        sk = skp.tile([128, QG * 128], BF16, tag="sk")
                        for j in range(QG):
                            q = g * QG + j
                            eng = sk_engine()
                            if eng is nc.scalar:
                                eng.mul(
                                    sk[:, j * 128:(j + 1) * 128],
                                    KV[:, c * 256 + 128:(c + 1) * 256],
                                    ATf[:, c * NQ + q:c * NQ + q + 1],
                                )
                            else:
                                eng.tensor_scalar_mul(
                                    sk[:, j * 128:(j + 1) * 128],
                                    KV[:, c * 256 + 128:(c + 1) * 256],
                                    ATf[:, c * NQ + q:c * NQ + q + 1],
                                )
                        nc.tensor.matmul(ps_j[:, 0:512],
                                         KV[:, c * 256:c * 256 + 128],
                                         sk[:, 0:512],
                                         start=(c == 0), stop=(c == C - 1))
                        nc.tensor.matmul(ps_j[:, 512:1024],
                                         KV[:, c * 256:c * 256 + 128],
                                         sk[:, 512:1024],
                                         start=(c == 0), stop=(c == C - 1))
                    jsb = jsbp.tile([128, QG * 128], F32, tag="jsb")
                    for j in range(QG):
                        q = g * QG + j
                        wb = wbp.tile([128, 128], F32, tag="wb")
                        nc.sync.dma_start(
                            wb[:, :],
                            wsb[q:q + 1, :].unsqueeze(1)
                            .broadcast_to((1, 128, 128)),
                        )
                        nc.vector.scalar_tensor_tensor(
                            jsb[:, j * 128:(j + 1) * 128],
                            wb[:, :],
                            negO[:, q:q + 1],
                            ps_j[:, j * 128:(j + 1) * 128],
                            ALU.mult, ALU.add,
                        )
                    nc.sync.dma_start(
                        out[b, g * QG:(g + 1) * QG].rearrange("j v k -> v j k"),
                        jsb[:, :].rearrange("v (j k) -> v j k", k=128),
                    )

    nc.compile()
    return nc


def _get_nc():
    if "nc" not in _CACHED:
        _CACHED["nc"] = _build()
    return _CACHED["nc"]


def _prep_core_inputs(query, keys, values, i):
    s = slice(i * BPC, (i + 1) * BPC)
    K = np.ascontiguousarray(keys[s])     # (2, 4096, 128) f32
    V = np.ascontiguousarray(values[s])
    Q = np.ascontiguousarray(query[s])    # (2, 64, 128) f32
    kvb = np.empty((BPC, C, 128, 256), dtype=ml_dtypes.bfloat16)
    kvb[:, :, :, 0:128] = V.reshape(BPC, C, 128, 128)
    kvb[:, :, :, 128:256] = K.reshape(BPC, C, 128, 128)
    kt = np.ascontiguousarray(K.transpose(0, 2, 1)).astype(np.float32)
    qt = np.ascontiguousarray(Q.transpose(0, 2, 1)).astype(np.float32)
    return {"kvb": kvb, "kt": kt, "qt": qt}


def kernel(query, keys, values):
    query = np.asarray(query, dtype=np.float32)
    keys = np.asarray(keys, dtype=np.float32)
    values = np.asarray(values, dtype=np.float32)
    nc = _get_nc()
    in_maps = [_prep_core_inputs(query, keys, values, i) for i in range(N_CORES)]
    res = run_bass_kernel_spmd(nc, in_maps, core_ids=list(range(N_CORES)))
    return np.concatenate([res.results[i]["out"] for i in range(N_CORES)],
                          axis=0).astype(np.float32)



# revision 2
# speedup vs baseline: 1.0550x; 1.0550x over previous
"""AttentionJacobian kernel for 8 TRN2 NeuronCores — v2.

J[b,q] = SCALE * ( V^T diag(a_q) K  -  o_q w_q^T ),  a = softmax(SCALE Q K^T)

Data-parallel over batch: 16 batches -> 2 per core. Per batch:
  scoresT chunks (n x q) = KT_c^T @ QT      (f32 matmuls, 8 chunks/psum bank)
  E = exp(SCALE * scoresT)                  (Act, one op per 512 cols, bf16)
  Zrow (1,nq) and Zq (nq,1) via ones-matmuls; rzb = SCALE/Z bcast (PE)
  ow rows (q-part) = [E^T V | E^T K]        (one 256-col matmul per chunk)
  o half scaled by -SCALE/Z^2 during psum evacuation; DRAM round-trip
  moves ow rows to partition 0 for rank-1 term2 matmuls.
  per group g of 8 q's: psum <- rank-1 -o_q w_q^T, then accumulate
  32 chunks of V_c^T @ (a ⊙ K_c); sk tiles produced by DVE (dual-scalar
  tensor_scalar from E and rzb; 5/8), Act (1.5/8), Pool (1.5/8).
  Evacuate psum via Act copy, DMA to DRAM.
"""

import sys

for p in ("/opt/trn_rl_repo",):
    if p not in sys.path:
        sys.path.append(p)

import numpy as np
import ml_dtypes

import concourse.bass as bass
import concourse.bacc as bacc
import concourse.tile as tile
from concourse import mybir
from concourse.bass_utils import run_bass_kernel_spmd

N_CORES = 8
BATCH = 16
NQ = 64
SEQ = 4096
D = 128
BPC = BATCH // N_CORES        # batches per core = 2
C = SEQ // 128                # 32 contraction chunks
QG = 8                        # q per output group
NG = NQ // QG                 # 8 groups
SCALE = float(D) ** -0.5

F32 = mybir.dt.float32
BF16 = mybir.dt.bfloat16
AF = mybir.ActivationFunctionType
ALU = mybir.AluOpType

_CACHED = {}


def _build():
    nc = bacc.Bacc("TRN2", target_bir_lowering=False, debug=False,
                   num_devices=N_CORES)

    kvb = nc.dram_tensor("kvb", [BPC, C, 128, 256], BF16, kind="ExternalInput").ap()
    kt = nc.dram_tensor("kt", [BPC, 128, SEQ], BF16, kind="ExternalInput").ap()
    qt = nc.dram_tensor("qt", [BPC, 128, NQ], BF16, kind="ExternalInput").ap()
    out = nc.dram_tensor("out", [BPC, NQ, D, D], F32, kind="ExternalOutput").ap()

    with tile.TileContext(nc) as tc:
        with (
            tc.tile_pool(name="const", bufs=1) as constp,
            tc.tile_pool(name="kv", bufs=2) as kvp,
            tc.tile_pool(name="ktp", bufs=2) as ktp,
            tc.tile_pool(name="qtp", bufs=2) as qtp,
            tc.tile_pool(name="ep", bufs=2) as ep,
            tc.tile_pool(name="rzp", bufs=2) as rzp,
            tc.tile_pool(name="atp", bufs=2) as atp,
            tc.tile_pool(name="owp", bufs=2) as owp,
            tc.tile_pool(name="skp", bufs=8) as skp,
            tc.tile_pool(name="jsbp", bufs=3) as jsbp,
            tc.tile_pool(name="owdram", bufs=2, space="DRAM") as owdp,
            tc.tile_pool(name="psj", bufs=2, space="PSUM") as psjp,
            tc.tile_pool(name="pss", bufs=2, space="PSUM") as pssp,
            tc.tile_pool(name="psmall", bufs=2, space="PSUM") as psmp,
        ):
            onescol = constp.tile([128, 1], BF16)
            nc.vector.memset(onescol[:, :], 1.0)
            onesrowS = constp.tile([1, 128], F32)
            nc.vector.memset(onesrowS[:, :], SCALE)

            it_ctr = [0]

            def sk_split():
                i = it_ctr[0] % 12
                it_ctr[0] += 1
                n_dve = 4 if i == 11 else 5
                n_act = 2 if i in (0, 2, 5, 7, 9) else 1
                return n_dve, n_act

            def head(b, st):
                """Per-batch prologue, 5 pieces (yield between each)."""
                QT = qtp.tile([128, NQ], BF16, tag="qt")
                nc.sync.dma_start(QT[:, :], qt[b])
                KT = ktp.tile([128, SEQ], BF16, tag="kt")
                for kc in range(4):
                    nc.sync.dma_start(KT[:, kc * 1024:(kc + 1) * 1024],
                                      kt[b][:, kc * 1024:(kc + 1) * 1024])
                KV = kvp.tile([128, C * 256], BF16, tag="kv")
                nc.sync.dma_start(KV[:, :].rearrange("p (c j) -> p c j", j=256),
                                  kvb[b].rearrange("c n j -> n c j"))
                st["KV"] = KV
                E = ep.tile([128, C * NQ], BF16, tag="e")
                st["E"] = E
                yield

                for cs in range(C // 8):
                    ps_s = pssp.tile([128, 8 * NQ], F32, tag="scores")
                    for c8 in range(8):
                        c = cs * 8 + c8
                        nc.tensor.matmul(ps_s[:, c8 * NQ:(c8 + 1) * NQ],
                                         KT[:, c * 128:(c + 1) * 128],
                                         QT[:, :], start=True, stop=True)
                    nc.scalar.activation(E[:, cs * 8 * NQ:(cs + 1) * 8 * NQ],
                                         ps_s[:, :], AF.Exp, bias=0.0,
                                         scale=SCALE)
                    if cs == 1:
                        yield

                # one psum bank for all small outputs:
                # [0:1,0:64] Zrow | [:,64:128] rzb | [0:64,128:129] Zq |
                # [0:64,256:512] ow
                ps_sm = psmp.tile([128, 512], F32, tag="small")
                ps_z = ps_sm[0:1, 0:NQ]
                for c in range(C):
                    nc.tensor.matmul(ps_z, onescol[:, :],
                                     E[:, c * NQ:(c + 1) * NQ],
                                     start=(c == 0), stop=(c == C - 1))
                rz = rzp.tile([1, NQ], F32, tag="rz")
                nc.vector.reciprocal(rz[:, :], ps_z)
                ps_rzb = ps_sm[:, NQ:2 * NQ]
                nc.tensor.matmul(ps_rzb, onesrowS[:, :], rz[:, :],
                                 start=True, stop=True)
                rzb = rzp.tile([128, NQ], F32, tag="rzbsb")
                nc.scalar.copy(rzb[:, :], ps_rzb)
                ATf = atp.tile([128, C * NQ], F32, tag="atf")
                st["ATf"] = ATf
                for g in range(NG):
                    s = g * QG
                    nc.vector.tensor_mul(
                        ATf[:, :].rearrange("p (c q) -> p c q", q=NQ)[:, :, s:s + QG],
                        E[:, :].rearrange("p (c q) -> p c q", q=NQ)[:, :, s:s + QG],
                        rzb[:, s:s + QG].unsqueeze(1).broadcast_to((128, C, QG)),
                    )
                yield

                # Zq (NQ,1) -> m_o = -SCALE / Zq^2
                ps_zq = ps_sm[0:NQ, 128:129]
                for c in range(C):
                    nc.tensor.matmul(ps_zq, E[:, c * NQ:(c + 1) * NQ],
                                     onescol[:, :],
                                     start=(c == 0), stop=(c == C - 1))
                zq = rzp.tile([NQ, 1], F32, tag="zqsb")
                nc.vector.tensor_copy(zq[:, :], ps_zq)
                rq = rzp.tile([NQ, 1], F32, tag="rqsb")
                nc.vector.reciprocal(rq[:, :], zq[:, :])
                m_o = rzp.tile([NQ, 1], F32, tag="mo")
                nc.vector.scalar_tensor_tensor(m_o[:, :], rq[:, :], -SCALE,
                                               rq[:, :], ALU.mult, ALU.mult)
                yield

                # ow rows [E^T V | E^T K]; scale o by m_o; round-trip via DRAM
                ps_ow = ps_sm[0:NQ, 256:512]
                for c in range(C):
                    nc.tensor.matmul(ps_ow, E[:, c * NQ:(c + 1) * NQ],
                                     KV[:, c * 256:(c + 1) * 256],
                                     start=(c == 0), stop=(c == C - 1))
                owsb = owp.tile([NQ, 256], BF16, tag="owsb")
                nc.scalar.mul(owsb[:, 0:128], ps_ow[:, 0:128], m_o[:, 0:1])
                nc.scalar.copy(owsb[:, 128:256], ps_ow[:, 128:256])
                owd = owdp.tile([NQ, 256], BF16, tag="owd")
                nc.sync.dma_start(owd[:, :], owsb[:, :])
                owflat = owp.tile([1, NQ * 256], BF16, tag="owflat")
                nc.sync.dma_start(owflat[:, :],
                                  owd[:, :].rearrange("q m -> (q m)").unsqueeze(0))
                st["owflat"] = owflat
                yield

            def term1(b, st):
                """Per-batch main loop; yields after each of NG groups."""
                KV, ATf, E = st["KV"], st["ATf"], st["E"]
                for g in range(NG):
                    ps_j = psjp.tile([128, QG * 128], F32, tag="j")
                    for c in range(C):
                        sk = skp.tile([128, QG * 128], BF16, tag="sk")
                        kslice = KV[:, c * 256 + 128:(c + 1) * 256]
                        n_dve, n_act = sk_split()
                        for j in range(QG):
                            q = g * QG + j
                            acol = ATf[:, c * NQ + q:c * NQ + q + 1]
                            dst = sk[:, j * 128:(j + 1) * 128]
                            if j < n_dve:
                                nc.vector.tensor_scalar_mul(dst, kslice, acol)
                            elif j < n_dve + n_act:
                                nc.scalar.mul(dst, kslice, acol)
                            else:
                                nc.gpsimd.tensor_scalar_mul(dst, kslice, acol)
                        nc.tensor.matmul(ps_j[:, 0:512],
                                         KV[:, c * 256:c * 256 + 128],
                                         sk[:, 0:512],
                                         start=(c == 0), stop=False,
                                         skip_group_check=True)
                        nc.tensor.matmul(ps_j[:, 512:1024],
                                         KV[:, c * 256:c * 256 + 128],
                                         sk[:, 512:1024],
                                         start=(c == 0), stop=False,
                                         skip_group_check=True)
                    owflat = st["owflat"]
                    for j in range(QG):
                        q = g * QG + j
                        nc.tensor.matmul(
                            ps_j[:, j * 128:(j + 1) * 128],
                            owflat[0:1, q * 256:q * 256 + 128],
                            owflat[0:1, q * 256 + 128:(q + 1) * 256],
                            start=False, stop=True, skip_group_check=True)
                    jsb = jsbp.tile([128, QG * 128], F32, tag="jsb")
                    nc.scalar.copy(jsb[:, :], ps_j[:, :])
                    nc.sync.dma_start(
                        out[b, g * QG:(g + 1) * QG].rearrange("j v k -> v j k"),
                        jsb[:, :].rearrange("v (j k) -> v j k", k=128),
                    )
                    yield

            states = [{} for _ in range(BPC)]
            heads = [head(b, states[b]) for b in range(BPC)]
            terms = [term1(b, states[b]) for b in range(BPC)]
            for _ in heads[0]:
                pass
            for b in range(BPC):
                nxt = heads[b + 1] if b + 1 < BPC else None
                for g in range(NG):
                    next(terms[b], None)
                    if nxt is not None:
                        next(nxt, None)

    nc.compile()
    return nc


def _get_nc():
    if "nc" not in _CACHED:
        _CACHED["nc"] = _build()
    return _CACHED["nc"]


def _prep_core_inputs(query, keys, values, i):
    s = slice(i * BPC, (i + 1) * BPC)
    K = np.ascontiguousarray(keys[s])     # (2, 4096, 128) f32
    V = np.ascontiguousarray(values[s])
    Q = np.ascontiguousarray(query[s])    # (2, 64, 128) f32
    kvb = np.empty((BPC, C, 128, 256), dtype=ml_dtypes.bfloat16)
    kvb[:, :, :, 0:128] = V.reshape(BPC, C, 128, 128)
    kvb[:, :, :, 128:256] = K.reshape(BPC, C, 128, 128)
    kt = np.ascontiguousarray(K.transpose(0, 2, 1)).astype(ml_dtypes.bfloat16)
    qt = np.ascontiguousarray(Q.transpose(0, 2, 1)).astype(ml_dtypes.bfloat16)
    return {"kvb": kvb, "kt": kt, "qt": qt}


def kernel(query, keys, values):
    query = np.asarray(query, dtype=np.float32)
    keys = np.asarray(keys, dtype=np.float32)
    values = np.asarray(values, dtype=np.float32)
    nc = _get_nc()
    in_maps = [_prep_core_inputs(query, keys, values, i) for i in range(N_CORES)]
    res = run_bass_kernel_spmd(nc, in_maps, core_ids=list(range(N_CORES)))
    return np.concatenate([res.results[i]["out"] for i in range(N_CORES)],
                          axis=0).astype(np.float32)
